# revision 15
# baseline (speedup 1.0000x reference)
"""Trainium2 Bass kernel for EnergyAwareAdaptiveFusion (moe_routing).

Strategy:
  - Only rows with route_choice == 2 need the "full" fusion branch; rows with
    route_choice 0/1 are exact copies of img_emb/txt_emb (assembled on host).
  - The selected rows are gathered, padded to a multiple of 8*384, and
    data-parallel sharded across the 8 NeuronCores (replicated params).
  - On-device everything is computed feature-major ([feature_part, row_free]) so
    every GEMM chains without transposes, and all per-feature biases map to
    per-partition scalars.
  - Matmuls run in fp32r (fp32 with 11-bit mantissa): full-rate on the PE with
    ~1e-4 input rounding error only.  Inputs/weights are pre-rounded on host;
    on-device intermediates are written as fp32r by DVE/ACT.
  - seq-len-2 attention: softmax over 2 logits == sigmoid of the scaled score
    difference; ctx-mean collapses Wo to a single GEMM on the mean context.
"""
import numpy as np

import concourse.mybir as mybir
import concourse.tile as tile
from concourse import bacc
from concourse.bass_utils import run_bass_kernel_spmd

P = 128
D = 1024
NF = D // P          # 8 feature tiles
H = 16
NB = 384             # rows per block (PSUM-bank friendly, fp32r full rate)
NCORES = 8
EPS = 1e-5

f32 = mybir.dt.float32
f32r = mybir.dt.float32r

Act = mybir.ActivationFunctionType
Alu = mybir.AluOpType

GELU_FUNC = Act.Gelu  # test_sim.py swaps to Identity (CoreSim lacks Gelu)


def _round_f32r(x):
    """Round fp32 -> fp32r (11-bit mantissa, RNE); matches walrus fp32_to_fp32r."""
    u = np.ascontiguousarray(x, dtype=np.float32).view(np.uint32)
    lsb = (u >> 12) & 1
    r = (u.astype(np.uint64) + 0x7FF + lsb) & 0xFFFFF000
    return r.astype(np.uint32).view(np.float32)


def _pack_w(w):
    """[K, M] -> [128, M/128, K/128, 128]: per m-column, k-major, contiguous."""
    K, M = w.shape
    nk, nm = K // P, M // P
    return np.ascontiguousarray(
        _round_f32r(w).reshape(nk, P, nm, P).transpose(1, 2, 0, 3))


def _pack_b(b):
    """[M] -> [128, M/128] per-partition bias layout."""
    return np.ascontiguousarray(b.reshape(-1, P).T.astype(np.float32))


def _build(nblocks, reps=1, stop_after=None):
    """Build the per-core program for R = nblocks*NB rows.

    stop_after: debug bisect hook — "A" (gate/blend), "B" (attention),
    "W" (Wo+residual), "L" (layernorm); None = full pipeline.
    """
    R = nblocks * NB
    nc = bacc.Bacc(target_bir_lowering=False, debug=False)

    img_d = nc.dram_tensor("img", [P, NF, R], f32r, kind="ExternalInput")
    txt_d = nc.dram_tensor("txt", [P, NF, R], f32r, kind="ExternalInput")
    wg_d = nc.dram_tensor("wg", [P, 8, 16, P], f32r, kind="ExternalInput")
    wqkv_d = nc.dram_tensor("wqkv", [P, 24, 8, P], f32r, kind="ExternalInput")
    wo_d = nc.dram_tensor("wo", [P, 8, 8, P], f32r, kind="ExternalInput")
    wf1_d = nc.dram_tensor("wf1", [P, 32, 8, P], f32r, kind="ExternalInput")
    wf2_d = nc.dram_tensor("wf2", [P, 8, 32, P], f32r, kind="ExternalInput")
    bg_d = nc.dram_tensor("bg", [P, 8], f32, kind="ExternalInput")
    bqkv_d = nc.dram_tensor("bqkv", [P, 24], f32, kind="ExternalInput")
    bo_d = nc.dram_tensor("bo", [P, 8], f32, kind="ExternalInput")
    bf1_d = nc.dram_tensor("bf1", [P, 32], f32, kind="ExternalInput")
    bf2_d = nc.dram_tensor("bf2", [P, 8], f32, kind="ExternalInput")
    gamma_d = nc.dram_tensor("gamma", [P, 8], f32, kind="ExternalInput")
    beta_d = nc.dram_tensor("beta", [P, 8], f32, kind="ExternalInput")
    mask_d = nc.dram_tensor("mask", [P, 2], f32r, kind="ExternalInput")
    bmaskh_d = nc.dram_tensor("bmaskh", [2, P], f32r, kind="ExternalInput")
    ones2_d = nc.dram_tensor("ones2", [P, 2], f32r, kind="ExternalInput")
    ones1_d = nc.dram_tensor("ones1", [1, P], f32r, kind="ExternalInput")
    out_d = nc.dram_tensor("out", [P, NF, R], f32, kind="ExternalOutput")

    import contextlib
    with tile.TileContext(nc) as tc, contextlib.ExitStack() as ctx:
        ctx.enter_context(nc.allow_low_precision(
            reason="fp32r (11-bit-mantissa) rounding of PE inputs is intentional"))
        if True:
            # NOTE: ph bufs=2 is required — a single h buffer reused across
            # row-blocks faults the device (NRT_EXEC_UNIT_UNRECOVERABLE).
            consts = ctx.enter_context(tc.tile_pool(name="consts", bufs=1))
            pio = ctx.enter_context(tc.tile_pool(name="pio", bufs=1))
            pfused = ctx.enter_context(tc.tile_pool(name="pfused", bufs=2))
            pctx = ctx.enter_context(tc.tile_pool(name="pctx", bufs=1))
            ph = ctx.enter_context(tc.tile_pool(name="ph", bufs=2))
            pw = ctx.enter_context(tc.tile_pool(name="pw", bufs=4))
            pt = ctx.enter_context(tc.tile_pool(name="pt", bufs=6))
            pq = ctx.enter_context(tc.tile_pool(name="pq", bufs=8))
            psm = ctx.enter_context(tc.tile_pool(name="psm", bufs=7))
            psum = ctx.enter_context(tc.tile_pool(name="psum", bufs=4, space="PSUM"))
            pss = ctx.enter_context(tc.tile_pool(name="pss", bufs=2, space="PSUM"))
            psb = ctx.enter_context(tc.tile_pool(name="psb", bufs=2, space="PSUM"))

            # constants / params (load once)
            bg_sb = consts.tile([P, 8], f32)
            nc.sync.dma_start(out=bg_sb, in_=bg_d[:, :])
            bqkv_sb = consts.tile([P, 24], f32)
            nc.sync.dma_start(out=bqkv_sb, in_=bqkv_d[:, :])
            bo_sb = consts.tile([P, 8], f32)
            nc.sync.dma_start(out=bo_sb, in_=bo_d[:, :])
            bf1_sb = consts.tile([P, 32], f32)
            nc.sync.dma_start(out=bf1_sb, in_=bf1_d[:, :])
            bf2_sb = consts.tile([P, 8], f32)
            nc.sync.dma_start(out=bf2_sb, in_=bf2_d[:, :])
            gamma_sb = consts.tile([P, 8], f32)
            nc.sync.dma_start(out=gamma_sb, in_=gamma_d[:, :])
            beta_sb = consts.tile([P, 8], f32)
            nc.sync.dma_start(out=beta_sb, in_=beta_d[:, :])
            mask_sb = consts.tile([P, 2], f32r)
            nc.sync.dma_start(out=mask_sb, in_=mask_d[:, :])
            bmaskh_sb = consts.tile([2, P], f32r)
            nc.sync.dma_start(out=bmaskh_sb, in_=bmaskh_d[:, :])
            ones2_sb = consts.tile([P, 2], f32r)
            nc.sync.dma_start(out=ones2_sb, in_=ones2_d[:, :])
            ones1_sb = consts.tile([1, P], f32r)
            nc.sync.dma_start(out=ones1_sb, in_=ones1_d[:, :])
            eps_sb = consts.tile([1, 1], f32)
            nc.vector.memset(eps_sb, EPS)

            def emit_block(b):
                bs, be = b * NB, (b + 1) * NB

                def dump(src_f32_ap_by_m, nf=NF):
                    for m in range(nf):
                        o_t = pt.tile([P, NB], f32, tag="t")
                        nc.vector.tensor_copy(o_t, src_f32_ap_by_m(m))
                        nc.sync.dma_start(out=out_d[:, m % NF, bs:be], in_=o_t)

                img_sb = pio.tile([P, NF, NB], f32r, tag="img")
                nc.sync.dma_start(out=img_sb, in_=img_d[:, :, bs:be])
                txt_sb = pio.tile([P, NF, NB], f32r, tag="txt")
                nc.sync.dma_start(out=txt_sb, in_=txt_d[:, :, bs:be])
                img_f = img_sb.bitcast(f32)
                txt_f = txt_sb.bitcast(f32)

                fused_sb = pfused.tile([P, NF, NB], f32r, tag="fused")
                fused_f = fused_sb.bitcast(f32)

                # ---- stage A: gate = sigmoid([img|txt] @ Wg + bg); blend
                for m in range(NF):
                    ps = psum.tile([P, NB], f32, tag="mm")
                    for ck in range(2):
                        w_sb = pw.tile([P, 8, P], f32r, tag="w")
                        nc.sync.dma_start(
                            out=w_sb, in_=wg_d[:, m, ck * 8:(ck + 1) * 8, :])
                        src = img_sb if ck == 0 else txt_sb
                        for k in range(8):
                            nc.tensor.matmul(
                                ps, w_sb[:, k, :], src[:, k, :],
                                start=(ck == 0 and k == 0),
                                stop=(ck == 1 and k == 7))
                    gate_t = pt.tile([P, NB], f32, tag="t")
                    nc.scalar.activation(gate_t, ps, Act.Sigmoid,
                                         bias=bg_sb[:, m:m + 1])
                    d_t = pt.tile([P, NB], f32, tag="t")
                    nc.vector.tensor_sub(d_t, img_f[:, m, :], txt_f[:, m, :])
                    nc.vector.tensor_mul(d_t, gate_t, d_t)
                    nc.vector.tensor_add(fused_sb[:, m, :], d_t, txt_f[:, m, :])

                if stop_after == "A":
                    dump(lambda m: fused_f[:, m, :])
                    return

                # ---- stage B: qkv + seq-2 attention -> mean context
                ctx_sb = pctx.tile([P, NF, NB], f32r, tag="ctx")
                for fi in range(NF):
                    wq = pw.tile([P, 8, P], f32r, tag="w")
                    nc.sync.dma_start(out=wq, in_=wqkv_d[:, fi, :, :])
                    wk = pw.tile([P, 8, P], f32r, tag="w")
                    nc.sync.dma_start(out=wk, in_=wqkv_d[:, 8 + fi, :, :])
                    ps_q0 = psum.tile([P, NB], f32, tag="mm")
                    ps_q1 = psum.tile([P, NB], f32, tag="mm")
                    ps_k0 = psum.tile([P, NB], f32, tag="mm")
                    ps_k1 = psum.tile([P, NB], f32, tag="mm")
                    for k in range(8):
                        st, sp = (k == 0), (k == 7)
                        nc.tensor.matmul(ps_q0, wq[:, k, :], img_sb[:, k, :],
                                         start=st, stop=sp)
                        nc.tensor.matmul(ps_q1, wq[:, k, :], txt_sb[:, k, :],
                                         start=st, stop=sp)
                        nc.tensor.matmul(ps_k0, wk[:, k, :], img_sb[:, k, :],
                                         start=st, stop=sp)
                        nc.tensor.matmul(ps_k1, wk[:, k, :], txt_sb[:, k, :],
                                         start=st, stop=sp)
                    q0 = pq.tile([P, NB], f32, tag="q")
                    nc.vector.tensor_scalar(q0, ps_q0, bqkv_sb[:, fi:fi + 1],
                                            None, Alu.add)
                    q1 = pq.tile([P, NB], f32, tag="q")
                    nc.vector.tensor_scalar(q1, ps_q1, bqkv_sb[:, fi:fi + 1],
                                            None, Alu.add)
                    k0t = pq.tile([P, NB], f32, tag="q")
                    nc.vector.tensor_copy(k0t, ps_k0)
                    kd = pq.tile([P, NB], f32, tag="q")
                    nc.vector.tensor_tensor(kd, k0t, ps_k1, Alu.subtract)
                    tmp0 = pq.tile([P, NB], f32r, tag="q")
                    nc.vector.tensor_mul(tmp0, q0, kd)
                    tmp1 = pq.tile([P, NB], f32r, tag="q")
                    nc.vector.tensor_mul(tmp1, q1, kd)
                    ps_d0 = pss.tile([2, NB], f32, tag="st")
                    nc.tensor.matmul(ps_d0, mask_sb, tmp0, start=True, stop=True)
                    ps_d1 = pss.tile([2, NB], f32, tag="st")
                    nc.tensor.matmul(ps_d1, mask_sb, tmp1, start=True, stop=True)
                    a0 = psm.tile([2, NB], f32, tag="sc")
                    nc.scalar.activation(a0, ps_d0, Act.Sigmoid, scale=0.125)
                    a1 = psm.tile([2, NB], f32, tag="sc")
                    nc.scalar.activation(a1, ps_d1, Act.Sigmoid, scale=0.125)
                    asum = psm.tile([2, NB], f32r, tag="sc")
                    nc.vector.tensor_add(asum, a0, a1)

                    wv = pw.tile([P, 8, P], f32r, tag="w")
                    nc.sync.dma_start(out=wv, in_=wqkv_d[:, 16 + fi, :, :])
                    ps_v0 = psum.tile([P, NB], f32, tag="mm")
                    ps_v1 = psum.tile([P, NB], f32, tag="mm")
                    for k in range(8):
                        st, sp = (k == 0), (k == 7)
                        nc.tensor.matmul(ps_v0, wv[:, k, :], img_sb[:, k, :],
                                         start=st, stop=sp)
                        nc.tensor.matmul(ps_v1, wv[:, k, :], txt_sb[:, k, :],
                                         start=st, stop=sp)
                    v0t = pq.tile([P, NB], f32, tag="q")
                    nc.vector.tensor_scalar(v0t, ps_v0, bqkv_sb[:, 16 + fi:17 + fi],
                                            None, Alu.add)
                    v1t = pq.tile([P, NB], f32, tag="q")
                    nc.vector.tensor_scalar(v1t, ps_v1, bqkv_sb[:, 16 + fi:17 + fi],
                                            None, Alu.add)
                    vd = pq.tile([P, NB], f32, tag="q")
                    nc.vector.tensor_sub(vd, v0t, v1t)
                    ps_c = psb.tile([P, NB], f32, tag="bc")
                    nc.tensor.matmul(ps_c, bmaskh_sb, asum, start=True, stop=True)
                    ct = pt.tile([P, NB], f32, tag="t")
                    nc.vector.tensor_tensor(ct, vd, ps_c, Alu.mult)
                    nc.vector.tensor_add(ctx_sb[:, fi, :], ct, v1t)

                if stop_after == "B":
                    dump(lambda m: ctx_sb.bitcast(f32)[:, m, :])
                    return

                # ---- Wo on mean context; residual into fused
                for m in range(NF):
                    wo_sb = pw.tile([P, 8, P], f32r, tag="w")
                    nc.sync.dma_start(out=wo_sb, in_=wo_d[:, m, :, :])
                    ps = psum.tile([P, NB], f32, tag="mm")
                    for k in range(8):
                        nc.tensor.matmul(ps, wo_sb[:, k, :], ctx_sb[:, k, :],
                                         start=(k == 0), stop=(k == 7))
                    f2 = pt.tile([P, NB], f32, tag="t")
                    nc.vector.tensor_scalar(f2, ps, bo_sb[:, m:m + 1],
                                            None, Alu.add)
                    nc.vector.tensor_add(fused_sb[:, m, :], f2, fused_f[:, m, :])

                if stop_after == "W":
                    dump(lambda m: fused_f[:, m, :])
                    return

                # ---- LayerNorm (feature reduction via PE ones-matmul)
                ps_mu = pss.tile([2, NB], f32, tag="st")
                for m in range(NF):
                    nc.tensor.matmul(ps_mu, ones2_sb, fused_sb[:, m, :],
                                     start=(m == 0), stop=(m == NF - 1))
                ps_sq = pss.tile([2, NB], f32, tag="st")
                for m in range(NF):
                    x2 = pt.tile([P, NB], f32r, tag="t")
                    nc.vector.tensor_mul(x2, fused_f[:, m, :], fused_f[:, m, :])
                    nc.tensor.matmul(ps_sq, ones2_sb, x2,
                                     start=(m == 0), stop=(m == NF - 1))
                mean = psm.tile([1, NB], f32, tag="sc")
                nc.vector.tensor_scalar(mean, ps_mu[0:1, :], 1.0 / D, None,
                                        Alu.mult)
                ex2 = psm.tile([1, NB], f32, tag="sc")
                nc.vector.tensor_scalar(ex2, ps_sq[0:1, :], 1.0 / D, None,
                                        Alu.mult)
                var = psm.tile([1, NB], f32, tag="sc")
                nc.vector.tensor_mul(var, mean, mean)
                nc.vector.tensor_tensor(var, ex2, var, Alu.subtract)
                sd = psm.tile([1, NB], f32, tag="sc")
                nc.scalar.activation(sd, var, Act.Sqrt, bias=eps_sb[0:1, :])
                rs = psm.tile([1, NB], f32r, tag="sc")
                nc.vector.reciprocal(rs, sd)
                ms = psm.tile([1, NB], f32r, tag="sc")
                nc.vector.tensor_mul(ms, mean, rs.bitcast(f32))
                ps_rsb = psb.tile([P, NB], f32, tag="bc")
                nc.tensor.matmul(ps_rsb, ones1_sb, rs, start=True, stop=True)
                ps_msb = psb.tile([P, NB], f32, tag="bc")
                nc.tensor.matmul(ps_msb, ones1_sb, ms, start=True, stop=True)
                for m in range(NF):
                    t = pt.tile([P, NB], f32, tag="t")
                    nc.vector.tensor_tensor(t, fused_f[:, m, :], ps_rsb, Alu.mult)
                    t2 = pt.tile([P, NB], f32, tag="t")
                    nc.vector.tensor_tensor(t2, t, ps_msb, Alu.subtract)
                    nc.vector.tensor_scalar(fused_sb[:, m, :], t2,
                                            gamma_sb[:, m:m + 1],
                                            beta_sb[:, m:m + 1],
                                            Alu.mult, Alu.add)

                if stop_after == "L":
                    dump(lambda m: fused_f[:, m, :])
                    return

                # ---- FFN
                h_sb = ph.tile([P, 32, NB], f32r, tag="h")
                for m in range(32):
                    w1 = pw.tile([P, 8, P], f32r, tag="w")
                    nc.sync.dma_start(out=w1, in_=wf1_d[:, m, :, :])
                    ps = psum.tile([P, NB], f32, tag="mm")
                    for k in range(8):
                        nc.tensor.matmul(ps, w1[:, k, :], fused_sb[:, k, :],
                                         start=(k == 0), stop=(k == 7))
                    nc.scalar.activation(h_sb[:, m, :], ps, GELU_FUNC,
                                         bias=bf1_sb[:, m:m + 1])
                if stop_after == "F":
                    dump(lambda m: h_sb.bitcast(f32)[:, m, :])
                    return

                for m in range(NF):
                    ps = psum.tile([P, NB], f32, tag="mm")
                    for ck in range(4):
                        w2 = pw.tile([P, 8, P], f32r, tag="w")
                        nc.sync.dma_start(
                            out=w2, in_=wf2_d[:, m, ck * 8:(ck + 1) * 8, :])
                        for k in range(8):
                            nc.tensor.matmul(
                                ps, w2[:, k, :], h_sb[:, ck * 8 + k, :],
                                start=(ck == 0 and k == 0),
                                stop=(ck == 3 and k == 7))
                    o_t = pt.tile([P, NB], f32, tag="t")
                    nc.vector.tensor_scalar(o_t, ps, bf2_sb[:, m:m + 1],
                                            None, Alu.add)
                    nc.sync.dma_start(out=out_d[:, m, bs:be], in_=o_t)

            if reps == 1:
                for b in range(nblocks):
                    emit_block(b)
            else:
                with tc.For_i(0, reps, 1):
                    for b in range(nblocks):
                        emit_block(b)

    nc.compile()
    return nc


_programs = {}


def _get_program(nblocks, reps=1):
    key = (nblocks, reps)
    if key not in _programs:
        _programs[key] = _build(nblocks, reps)
    return _programs[key]


def _prep_in_maps(img2, txt2, weights, nblocks):
    """img2/txt2: [N2P, D] gathered+padded rows. Returns per-core in_maps."""
    R = nblocks * NB
    n2p = NCORES * R

    def to_fm(x):  # [n2p, D] -> [128, NF, n2p] feature-major
        return np.ascontiguousarray(
            _round_f32r(x).reshape(n2p, NF, P).transpose(2, 1, 0))

    img_fm = to_fm(img2)
    txt_fm = to_fm(txt2)

    jj = np.arange(2)[None, :]
    pp = np.arange(P)[:, None]
    mask = ((pp // 64) == jj).astype(np.float32)          # [128, 2]
    bmaskh = np.ascontiguousarray(0.5 * mask.T)           # [2, 128]
    ones2 = np.ones((P, 2), np.float32)
    ones1 = np.ones((1, P), np.float32)

    shared = dict(
        wg=weights["wg"], wqkv=weights["wqkv"], wo=weights["wo"],
        wf1=weights["wf1"], wf2=weights["wf2"],
        bg=weights["bg"], bqkv=weights["bqkv"], bo=weights["bo"],
        bf1=weights["bf1"], bf2=weights["bf2"],
        gamma=weights["gamma"], beta=weights["beta"],
        mask=mask, bmaskh=bmaskh, ones2=ones2, ones1=ones1,
    )
    in_maps = []
    for c in range(NCORES):
        m = dict(shared)
        m["img"] = np.ascontiguousarray(img_fm[:, :, c * R:(c + 1) * R])
        m["txt"] = np.ascontiguousarray(txt_fm[:, :, c * R:(c + 1) * R])
        in_maps.append(m)
    return in_maps


def _pack_weights(Wg, bg, Wqkv, bqkv, Wo, bo, gamma, beta, Wf1, bf1, Wf2, bf2):
    return dict(
        wg=_pack_w(np.asarray(Wg)), wqkv=_pack_w(np.asarray(Wqkv)),
        wo=_pack_w(np.asarray(Wo)), wf1=_pack_w(np.asarray(Wf1)),
        wf2=_pack_w(np.asarray(Wf2)),
        bg=_pack_b(np.asarray(bg)), bqkv=_pack_b(np.asarray(bqkv)),
        bo=_pack_b(np.asarray(bo)), bf1=_pack_b(np.asarray(bf1)),
        bf2=_pack_b(np.asarray(bf2)),
        gamma=_pack_b(np.asarray(gamma)), beta=_pack_b(np.asarray(beta)),
    )


def _run_device(img2, txt2, weights, nblocks, reps=1):
    nc = _get_program(nblocks, reps)
    in_maps = _prep_in_maps(img2, txt2, weights, nblocks)
    res = run_bass_kernel_spmd(nc, in_maps, list(range(NCORES)), trace=False)
    R = nblocks * NB
    n2p = NCORES * R
    # [128, NF, R] per core -> [n2p, D]
    full = np.empty((n2p, D), np.float32)
    for c in range(NCORES):
        o = res.results[c]["out"]                  # [128, NF, R]
        full[c * R:(c + 1) * R] = o.transpose(2, 1, 0).reshape(R, D)
    return full


def kernel(img_emb, txt_emb, route_choice, Wg, bg, Wqkv, bqkv, Wo, bo,
           gamma, beta, Wf1, bf1, Wf2, bf2):
    img_emb = np.asarray(img_emb, dtype=np.float32)
    txt_emb = np.asarray(txt_emb, dtype=np.float32)
    route_choice = np.asarray(route_choice)

    out = np.empty_like(img_emb)
    m0 = route_choice == 0
    m1 = route_choice == 1
    m2 = ~(m0 | m1)
    out[m0] = img_emb[m0]
    out[m1] = txt_emb[m1]

    idx2 = np.flatnonzero(m2)
    n2 = idx2.size
    if n2 == 0:
        return out

    nblocks = max(1, -(-n2 // (NCORES * NB)))
    n2p = NCORES * nblocks * NB
    img2 = np.zeros((n2p, D), np.float32)
    txt2 = np.zeros((n2p, D), np.float32)
    img2[:n2] = img_emb[idx2]
    txt2[:n2] = txt_emb[idx2]

    weights = _pack_weights(Wg, bg, Wqkv, bqkv, Wo, bo, gamma, beta,
                            Wf1, bf1, Wf2, bf2)
    full = _run_device(img2, txt2, weights, nblocks)
    out[idx2] = full[:n2]
    return out
